# revision 14
# baseline (speedup 1.0000x reference)
"""
Trainium2 Bass kernel for nn_CrossAttention_62027917689453.

Math (per batch b):
    q = rgb @ Wq                       (N, E)
    k = freq @ Wk                      (N, E)
    scores = q @ k.T / sqrt(E)         (N, N)
    attn = softmax(scores, axis=-1)
    attn_out = attn @ freq             (N, D)
    out = concat([rgb, 0.5 * attn_out], axis=-1)   (N, 2D)

(ifreq / Wv are dead inputs in the reference and are ignored.)

Sharding: data-parallel over batch — 8 batches onto 8 NeuronCores, one
independent (N, N) attention slab per core. Full inputs in, full output out.

Per-core kernel layout choices:
  - All matmuls contract over the partition dim, so activations are needed
    transposed (d on partitions).  rgbT / freqT blocks are produced with PE
    transposes (bf16, 1 cyc/row) against an identity matrix.
  - Scores are computed TRANSPOSED: sT[m, n] = sum_e kT[e,m]^T qT[e,n], which
    makes exp(sT) (layout [m, n]) directly usable as the stationary operand of
    the attention-output matmul U[n, d] = sum_m P[m,n]^T freq[m,d] with freq in
    its natural layout — no transposes of the (N, N) attention matrix.
  - Softmax skips max-subtraction (scores are O(5) for this problem's
    distribution — exp is safe in fp32) and the denominator is obtained with
    N=1 matmuls against a ones-vector, folded into the same PSUM accumulation
    loop; normalization multiplies by 0.5 * reciprocal(colsum) on VectorE.
  - Matmul operands are bf16 (fp32 PSUM accumulation).
"""

import numpy as np

import concourse.bass as bass
import concourse.mybir as mybir
import concourse.tile as tile
from concourse.tile import TileContext

from concourse.masks import make_identity

F32 = mybir.dt.float32
BF16 = mybir.dt.bfloat16

B = 8          # batches == cores
N = 2048       # sequence length (n and m)
D = 1024       # feature dim (d and e)
P = 128        # partitions
NT = N // P    # 16  row chunks
DC = D // P    # 8   feature chunks
NBLK = 512     # n-block width for the q/scores pipeline
NG = N // NBLK # 4   n-blocks
SUB = NBLK // P  # 4 row-chunks per n-block


def _split_multi_waits(nc: bass.Bass) -> int:
    """The walrus build in this container cannot encode multi-semaphore waits
    on several instruction structs (CTRL Drain, PSEUDO_DMA_DIRECT2D, ...):
    setupSyncWait throws an internal error.  Rewrite every instruction that
    carries more than one wait so the extra waits sit on standalone
    single-wait EventSemaphore instructions immediately before it."""
    n_split = 0
    for f in nc.m.functions:
        for blk in f.blocks:
            insts = blk.instructions
            new: list = []
            changed = False
            for inst in insts:
                si = inst.sync_info
                if si is not None and len(si.on_wait) > 1:
                    waits = list(si.on_wait)
                    for w in waits[:-1]:
                        n_split += 1
                        ev = mybir.InstEventSemaphore(
                            name=f"I-msw-{n_split}",
                            ins=[],
                            outs=[],
                            sync_info=mybir.SyncInfo(on_wait=[w], on_update=[]),
                        )
                        ev.engine = inst.engine
                        new.append(ev)
                    si.on_wait.clear()
                    si.on_wait.append(waits[-1])
                    changed = True
                new.append(inst)
            if changed:
                insts[:] = new
    return n_split


def build_program() -> bass.Bass:
    nc = bass.Bass()
    rgb = nc.declare_dram_parameter("rgb", [N, D], F32, isOutput=False)
    freq = nc.declare_dram_parameter("freq", [N, D], F32, isOutput=False)
    wq = nc.declare_dram_parameter("Wq", [D, D], F32, isOutput=False)
    wk = nc.declare_dram_parameter("Wk", [D, D], F32, isOutput=False)
    out = nc.declare_dram_parameter("out", [N, 2 * D], F32, isOutput=True)

    with TileContext(nc) as tc:
        with (
            tc.tile_pool(name="statics", bufs=1) as statics,
            tc.tile_pool(name="ld", bufs=4) as ldp,
            tc.tile_pool(name="bfp", bufs=2) as bfp,
            tc.tile_pool(name="col", bufs=2) as colp,
            tc.tile_pool(name="qtp", bufs=2) as qtp,
            tc.tile_pool(name="pblk", bufs=2) as pblkp,
            tc.tile_pool(name="outp", bufs=3) as outp,
            tc.tile_pool(name="small", bufs=8) as smallp,
            tc.tile_pool(name="ps", bufs=4, space="PSUM") as psp,
            tc.tile_pool(name="psu", bufs=2, space="PSUM") as psup,
        ):
            ident = statics.tile([P, P], BF16, tag="ident")
            make_identity(nc, ident)
            ones = statics.tile([P, 1], BF16, tag="ones")
            nc.vector.memset(ones, 1.0)

            wq_bf = statics.tile([P, DC, D], BF16, tag="wq")
            wk_bf = statics.tile([P, DC, D], BF16, tag="wk")
            freq_bf = statics.tile([P, NT, D], BF16, tag="freqbf")

            # DMA issue order is the critical-path order: the first PE work
            # (freqT transposes) needs the early freq chunks; kT needs Wk;
            # qT of block 0 needs rgb block 0 + Wq; remaining rgb blocks
            # stream inside the main loop.
            def load_freq(mc):
                t = ldp.tile([P, D], F32, tag="ld")
                nc.sync.dma_start(out=t, in_=freq[mc * P:(mc + 1) * P, :])
                nc.vector.tensor_copy(out=freq_bf[:, mc, :], in_=t)

            def load_wk(dc):
                t2 = ldp.tile([P, D], F32, tag="ld")
                nc.sync.dma_start(out=t2, in_=wk[dc * P:(dc + 1) * P, :])
                nc.vector.tensor_copy(out=wk_bf[:, dc, :], in_=t2)

            # interleave the first freq chunks (PE transpose feed) with Wk
            # (kT matmul feed) so both arrive as early as possible
            for i in range(4):
                load_freq(i)
                load_wk(2 * i)
                load_wk(2 * i + 1)
            for mc in range(4, 8):
                load_freq(mc)

            def load_rgb_group(ng, defer_passthrough=False):
                # load rgb chunks; write the rgb passthrough output half
                rgb_bf = bfp.tile([P, SUB, D], BF16, tag="rgbbf")
                fp32_chunks = []
                for s in range(SUB):
                    nchunk = ng * SUB + s
                    t = ldp.tile([P, D], F32, tag="ld")
                    nc.sync.dma_start(
                        out=t, in_=rgb[nchunk * P:(nchunk + 1) * P, :]
                    )
                    nc.vector.tensor_copy(out=rgb_bf[:, s, :], in_=t)
                    if defer_passthrough:
                        fp32_chunks.append(t)
                    else:
                        nc.sync.dma_start(
                            out=out[nchunk * P:(nchunk + 1) * P, 0:D], in_=t
                        )
                return rgb_bf, fp32_chunks

            rgb_bf0, rgb0_chunks = load_rgb_group(0, defer_passthrough=True)

            for dc in range(DC):
                t = ldp.tile([P, D], F32, tag="ld")
                nc.sync.dma_start(out=t, in_=wq[dc * P:(dc + 1) * P, :])
                nc.vector.tensor_copy(out=wq_bf[:, dc, :], in_=t)

            for mc in range(8, NT):
                load_freq(mc)

            # ng=0 passthrough writes issue after the critical-path loads
            for s, t in enumerate(rgb0_chunks):
                nc.sync.dma_start(out=out[s * P:(s + 1) * P, 0:D], in_=t)

            # --- kT[e, m] = Wk[d, e]^T  freqT[d, m]  (all m up front) ---
            # Emission order software-pipelines PE work: transposes of group
            # mg+1 are emitted before the kT matmuls of group mg, so the PE
            # has transpose work while Wk is still loading.
            kt_bf = statics.tile([P, DC, N], BF16, tag="kt")
            fcols = [None] * NG

            def emit_ft(mg):
                fcol = colp.tile([P, DC, NBLK], BF16, tag="col")
                for dc in range(DC):
                    ps_t = psp.tile([P, NBLK], BF16, tag="ps")
                    for s in range(SUB):
                        mc = mg * SUB + s
                        nc.tensor.transpose(
                            ps_t[:, s * P:(s + 1) * P],
                            freq_bf[:, mc, dc * P:(dc + 1) * P],
                            ident,
                        )
                    nc.vector.tensor_copy(out=fcol[:, dc, :], in_=ps_t)
                fcols[mg] = fcol

            def emit_kt(mg):
                fcol = fcols[mg]
                for et in range(DC):
                    ps_k = psp.tile([P, NBLK], F32, tag="ps")
                    for dc in range(DC):
                        nc.tensor.matmul(
                            ps_k,
                            wk_bf[:, dc, et * P:(et + 1) * P],
                            fcol[:, dc, :],
                            start=(dc == 0),
                            stop=(dc == DC - 1),
                        )
                    nc.scalar.copy(
                        out=kt_bf[:, et, mg * NBLK:(mg + 1) * NBLK], in_=ps_k
                    )

            emit_ft(0)
            emit_ft(1)
            emit_kt(0)
            emit_ft(2)
            emit_kt(1)
            emit_ft(3)
            emit_kt(2)
            emit_kt(3)

            # --- main loop over n-blocks (software pipelined) ---
            def emit_qt(rgb_bf):
                # rgbT columns for this n-block, then qT[e, nblk]
                rcol = colp.tile([P, DC, NBLK], BF16, tag="col")
                for dc in range(DC):
                    ps_t = psp.tile([P, NBLK], BF16, tag="ps")
                    for s in range(SUB):
                        nc.tensor.transpose(
                            ps_t[:, s * P:(s + 1) * P],
                            rgb_bf[:, s, dc * P:(dc + 1) * P],
                            ident,
                        )
                    nc.vector.tensor_copy(out=rcol[:, dc, :], in_=ps_t)
                qt = qtp.tile([P, DC, NBLK], BF16, tag="qt")
                for et in range(DC):
                    ps_q = psp.tile([P, NBLK], F32, tag="ps")
                    for dc in range(DC):
                        nc.tensor.matmul(
                            ps_q,
                            wq_bf[:, dc, et * P:(et + 1) * P],
                            rcol[:, dc, :],
                            start=(dc == 0),
                            stop=(dc == DC - 1),
                        )
                    nc.scalar.copy(out=qt[:, et, :], in_=ps_q)
                return qt

            qt_cur = emit_qt(rgb_bf0)
            for ng in range(NG):
                qt = qt_cur

                # scoresT[m, nblk] -> P = exp(scoresT / 32)
                p_blk = pblkp.tile([P, NT, NBLK], BF16, tag="pblk")
                for mt in range(NT):
                    ps_s = psp.tile([P, NBLK], F32, tag="ps")
                    for et in range(DC):
                        nc.tensor.matmul(
                            ps_s,
                            kt_bf[:, et, mt * P:(mt + 1) * P],
                            qt[:, et, :],
                            start=(et == 0),
                            stop=(et == DC - 1),
                        )
                    nc.scalar.activation(
                        out=p_blk[:, mt, :],
                        in_=ps_s,
                        func=mybir.ActivationFunctionType.Exp,
                        scale=1.0 / 32.0,
                    )

                # prefetch + transpose + project the NEXT n-block's q before
                # the long U phase, so the PE never stalls at the boundary
                if ng + 1 < NG:
                    rgb_bf_next = load_rgb_group(ng + 1)[0]
                    qt_cur = emit_qt(rgb_bf_next)

                # U[n, d] + colsum, then normalize and store
                for ntl in range(SUB):
                    n0 = ntl * P
                    ps_u = psup.tile([P, D], F32, tag="psu")
                    ps_cs = psp.tile([P, NBLK], F32, tag="ps")
                    for mc in range(NT):
                        lhs = p_blk[:, mc, n0:n0 + P]
                        nc.tensor.matmul(
                            ps_u[:, 0:NBLK], lhs, freq_bf[:, mc, 0:NBLK],
                            start=(mc == 0), stop=(mc == NT - 1),
                        )
                        nc.tensor.matmul(
                            ps_u[:, NBLK:D], lhs, freq_bf[:, mc, NBLK:D],
                            start=(mc == 0), stop=(mc == NT - 1),
                        )
                        nc.tensor.matmul(
                            ps_cs[:, 0:1], lhs, ones,
                            start=(mc == 0), stop=(mc == NT - 1),
                        )
                    rc = smallp.tile([P, 1], F32, tag="rc")
                    nc.vector.reciprocal(rc, ps_cs[:, 0:1])
                    ot = outp.tile([P, D], F32, tag="ot")
                    # out = (U * (1/colsum)) * 0.5   (fusion weight)
                    nc.vector.tensor_scalar(
                        out=ot, in0=ps_u, scalar1=rc, scalar2=0.5,
                        op0=mybir.AluOpType.mult, op1=mybir.AluOpType.mult,
                    )
                    row0 = ng * NBLK + n0
                    nc.sync.dma_start(out=out[row0:row0 + P, D:2 * D], in_=ot)

    _split_multi_waits(nc)
    return nc


_CACHE: dict = {}


def _get_program() -> bass.Bass:
    if "nc" not in _CACHE:
        _CACHE["nc"] = build_program()
    return _CACHE["nc"]


def _run(in_maps, trace=False, **kw):
    from concourse.bass_utils import run_bass_kernel_spmd

    nc = _get_program()
    return run_bass_kernel_spmd(nc, in_maps, list(range(B)), trace=trace, **kw)


def kernel(rgb, freq, ifreq=None, Wq=None, Wk=None, Wv=None, **_unused):
    rgb = np.asarray(rgb, dtype=np.float32)
    freq = np.asarray(freq, dtype=np.float32)
    Wq = np.ascontiguousarray(np.asarray(Wq, dtype=np.float32))
    Wk = np.ascontiguousarray(np.asarray(Wk, dtype=np.float32))
    in_maps = [
        {
            "rgb": np.ascontiguousarray(rgb[c]),
            "freq": np.ascontiguousarray(freq[c]),
            "Wq": Wq,
            "Wk": Wk,
        }
        for c in range(B)
    ]
    res = _run(in_maps, trace=False)
    return np.stack([res.results[c]["out"] for c in range(B)], axis=0)


# revision 18
# speedup vs baseline: 1.0553x; 1.0553x over previous
"""
Trainium2 Bass kernel for nn_CrossAttention_62027917689453.

Math (per batch b):
    q = rgb @ Wq                       (N, E)
    k = freq @ Wk                      (N, E)
    scores = q @ k.T / sqrt(E)         (N, N)
    attn = softmax(scores, axis=-1)
    attn_out = attn @ freq             (N, D)
    out = concat([rgb, 0.5 * attn_out], axis=-1)   (N, 2D)

(ifreq / Wv are dead inputs in the reference and are ignored.)

Sharding: data-parallel over batch — 8 batches onto 8 NeuronCores, one
independent (N, N) attention slab per core. Full inputs in, full output out.

Per-core kernel layout choices:
  - All matmuls contract over the partition dim, so activations are needed
    transposed (d on partitions).  rgbT / freqT blocks are produced with PE
    transposes (bf16, 1 cyc/row) against an identity matrix.
  - Scores are computed TRANSPOSED: sT[m, n] = sum_e kT[e,m]^T qT[e,n], which
    makes exp(sT) (layout [m, n]) directly usable as the stationary operand of
    the attention-output matmul U[n, d] = sum_m P[m,n]^T freq[m,d] with freq in
    its natural layout — no transposes of the (N, N) attention matrix.
  - Softmax skips max-subtraction (scores are O(5) for this problem's
    distribution — exp is safe in fp32) and the denominator is obtained with
    N=1 matmuls against a ones-vector, folded into the same PSUM accumulation
    loop; normalization multiplies by 0.5 * reciprocal(colsum) on VectorE.
  - Matmul operands are bf16 (fp32 PSUM accumulation).
"""

import numpy as np

import concourse.bass as bass
import concourse.mybir as mybir
import concourse.tile as tile
from concourse.tile import TileContext

from concourse.masks import make_identity

F32 = mybir.dt.float32
BF16 = mybir.dt.bfloat16

B = 8          # batches == cores
N = 2048       # sequence length (n and m)
D = 1024       # feature dim (d and e)
P = 128        # partitions
NT = N // P    # 16  row chunks
DC = D // P    # 8   feature chunks
NBLK = 512     # n-block width for the q/scores pipeline
NG = N // NBLK # 4   n-blocks
SUB = NBLK // P  # 4 row-chunks per n-block


def _split_multi_waits(nc: bass.Bass) -> int:
    """The walrus build in this container cannot encode multi-semaphore waits
    on several instruction structs (CTRL Drain, PSEUDO_DMA_DIRECT2D, ...):
    setupSyncWait throws an internal error.  Rewrite every instruction that
    carries more than one wait so the extra waits sit on standalone
    single-wait EventSemaphore instructions immediately before it."""
    n_split = 0
    for f in nc.m.functions:
        for blk in f.blocks:
            insts = blk.instructions
            new: list = []
            changed = False
            for inst in insts:
                si = inst.sync_info
                if si is not None and len(si.on_wait) > 1:
                    waits = list(si.on_wait)
                    for w in waits[:-1]:
                        n_split += 1
                        ev = mybir.InstEventSemaphore(
                            name=f"I-msw-{n_split}",
                            ins=[],
                            outs=[],
                            sync_info=mybir.SyncInfo(on_wait=[w], on_update=[]),
                        )
                        ev.engine = inst.engine
                        new.append(ev)
                    si.on_wait.clear()
                    si.on_wait.append(waits[-1])
                    changed = True
                new.append(inst)
            if changed:
                insts[:] = new
    return n_split


def build_program() -> bass.Bass:
    nc = bass.Bass()
    rgb = nc.declare_dram_parameter("rgb", [N, D], F32, isOutput=False)
    freq = nc.declare_dram_parameter("freq", [N, D], F32, isOutput=False)
    wq = nc.declare_dram_parameter("Wq", [D, D], F32, isOutput=False)
    wk = nc.declare_dram_parameter("Wk", [D, D], F32, isOutput=False)
    out = nc.declare_dram_parameter("out", [N, 2 * D], F32, isOutput=True)

    with TileContext(nc) as tc:
        with (
            tc.tile_pool(name="statics", bufs=1) as statics,
            tc.tile_pool(name="ld", bufs=4) as ldp,
            tc.tile_pool(name="bfp", bufs=2) as bfp,
            tc.tile_pool(name="col", bufs=2) as colp,
            tc.tile_pool(name="qtp", bufs=2) as qtp,
            tc.tile_pool(name="pblk", bufs=2) as pblkp,
            tc.tile_pool(name="outp", bufs=3) as outp,
            tc.tile_pool(name="small", bufs=8) as smallp,
            tc.tile_pool(name="ps", bufs=4, space="PSUM") as psp,
            tc.tile_pool(name="psu", bufs=2, space="PSUM") as psup,
        ):
            ident = statics.tile([P, P], BF16, tag="ident")
            make_identity(nc, ident)
            ones = statics.tile([P, 1], BF16, tag="ones")
            nc.vector.memset(ones, 1.0)

            wq_bf = statics.tile([P, DC, D], BF16, tag="wq")
            wk_bf = statics.tile([P, DC, D], BF16, tag="wk")
            freq_bf = statics.tile([P, NT, D], BF16, tag="freqbf")

            # DMA issue order is the critical-path order: the first PE work
            # (freqT transposes) needs the early freq chunks; kT needs Wk;
            # qT of block 0 needs rgb block 0 + Wq; remaining rgb blocks
            # stream inside the main loop.
            def load_freq(mc):
                t = ldp.tile([P, D], F32, tag="ld")
                nc.sync.dma_start(out=t, in_=freq[mc * P:(mc + 1) * P, :])
                nc.vector.tensor_copy(out=freq_bf[:, mc, :], in_=t)

            def load_wk(dc):
                t2 = ldp.tile([P, D], F32, tag="ld")
                nc.sync.dma_start(out=t2, in_=wk[dc * P:(dc + 1) * P, :])
                nc.vector.tensor_copy(out=wk_bf[:, dc, :], in_=t2)

            # the first freq chunks feed the PE transposes; Wk follows so the
            # dc-outer kT accumulation can start on wk[0] while later wk
            # chunks are still in flight
            for mc in range(4):
                load_freq(mc)
            for dc in range(DC):
                load_wk(dc)
            for mc in range(4, 8):
                load_freq(mc)

            def load_rgb_group(ng, defer_passthrough=False):
                # load rgb chunks; write the rgb passthrough output half
                rgb_bf = bfp.tile([P, SUB, D], BF16, tag="rgbbf")
                fp32_chunks = []
                for s in range(SUB):
                    nchunk = ng * SUB + s
                    t = ldp.tile([P, D], F32, tag="ld")
                    nc.sync.dma_start(
                        out=t, in_=rgb[nchunk * P:(nchunk + 1) * P, :]
                    )
                    nc.vector.tensor_copy(out=rgb_bf[:, s, :], in_=t)
                    if defer_passthrough:
                        fp32_chunks.append(t)
                    else:
                        nc.sync.dma_start(
                            out=out[nchunk * P:(nchunk + 1) * P, 0:D], in_=t
                        )
                return rgb_bf, fp32_chunks

            for mc in range(8, NT):
                load_freq(mc)

            rgb_bf0, rgb0_chunks = load_rgb_group(0, defer_passthrough=True)

            for dc in range(DC):
                t = ldp.tile([P, D], F32, tag="ld")
                nc.sync.dma_start(out=t, in_=wq[dc * P:(dc + 1) * P, :])
                nc.vector.tensor_copy(out=wq_bf[:, dc, :], in_=t)

            # ng=0 passthrough writes issue after the critical-path loads
            for s, t in enumerate(rgb0_chunks):
                nc.sync.dma_start(out=out[s * P:(s + 1) * P, 0:D], in_=t)

            # --- kT[e, m] = Wk[d, e]^T  freqT[d, m]  (all m up front) ---
            # Emission order software-pipelines PE work: transposes of group
            # mg+1 are emitted before the kT matmuls of group mg, so the PE
            # has transpose work while Wk is still loading.
            kt_bf = statics.tile([P, DC, N], BF16, tag="kt")
            fcols = [None] * NG

            def emit_ft(mg):
                fcol = colp.tile([P, DC, NBLK], BF16, tag="col")
                for dc in range(DC):
                    ps_t = psp.tile([P, NBLK], BF16, tag="ps")
                    for s in range(SUB):
                        mc = mg * SUB + s
                        nc.tensor.transpose(
                            ps_t[:, s * P:(s + 1) * P],
                            freq_bf[:, mc, dc * P:(dc + 1) * P],
                            ident,
                        )
                    nc.vector.tensor_copy(out=fcol[:, dc, :], in_=ps_t)
                fcols[mg] = fcol

            def emit_kt(mg):
                # dc-outer accumulation: all 8 PSUM banks hold one et-tile
                # accumulator each, so kT matmuls start as soon as wk[0] is
                # resident instead of waiting for all of Wk.  The 8
                # accumulators borrow both PSUM pools (2x [P,1024] + 4x
                # [P,512]).
                fcol = fcols[mg]
                acc_a = psup.tile([P, D], F32, tag="psu")
                acc_b = psup.tile([P, D], F32, tag="psu")
                accs = [
                    acc_a[:, 0:NBLK], acc_a[:, NBLK:D],
                    acc_b[:, 0:NBLK], acc_b[:, NBLK:D],
                ] + [
                    psp.tile([P, NBLK], F32, tag="ps", name=f"kt_acc_{mg}_{j}")
                    for j in range(4)
                ]
                for dc in range(DC):
                    for et in range(DC):
                        nc.tensor.matmul(
                            accs[et],
                            wk_bf[:, dc, et * P:(et + 1) * P],
                            fcol[:, dc, :],
                            start=(dc == 0),
                            stop=(dc == DC - 1),
                        )
                for et in range(DC):
                    nc.scalar.copy(
                        out=kt_bf[:, et, mg * NBLK:(mg + 1) * NBLK],
                        in_=accs[et],
                    )

            emit_ft(0)
            emit_ft(1)
            emit_kt(0)
            emit_ft(2)
            emit_kt(1)
            emit_ft(3)
            emit_kt(2)
            emit_kt(3)

            # --- main loop over n-blocks (software pipelined) ---
            def emit_qt(rgb_bf):
                # rgbT columns for this n-block, then qT[e, nblk]
                rcol = colp.tile([P, DC, NBLK], BF16, tag="col")
                for dc in range(DC):
                    ps_t = psp.tile([P, NBLK], BF16, tag="ps")
                    for s in range(SUB):
                        nc.tensor.transpose(
                            ps_t[:, s * P:(s + 1) * P],
                            rgb_bf[:, s, dc * P:(dc + 1) * P],
                            ident,
                        )
                    nc.vector.tensor_copy(out=rcol[:, dc, :], in_=ps_t)
                qt = qtp.tile([P, DC, NBLK], BF16, tag="qt")
                for et in range(DC):
                    ps_q = psp.tile([P, NBLK], F32, tag="ps")
                    for dc in range(DC):
                        nc.tensor.matmul(
                            ps_q,
                            wq_bf[:, dc, et * P:(et + 1) * P],
                            rcol[:, dc, :],
                            start=(dc == 0),
                            stop=(dc == DC - 1),
                        )
                    nc.scalar.copy(out=qt[:, et, :], in_=ps_q)
                return qt

            qt_cur = emit_qt(rgb_bf0)
            for ng in range(NG):
                qt = qt_cur

                # scoresT[m, nblk] -> P = exp(scoresT / 32)
                p_blk = pblkp.tile([P, NT, NBLK], BF16, tag="pblk")
                for mt in range(NT):
                    ps_s = psp.tile([P, NBLK], F32, tag="ps")
                    for et in range(DC):
                        nc.tensor.matmul(
                            ps_s,
                            kt_bf[:, et, mt * P:(mt + 1) * P],
                            qt[:, et, :],
                            start=(et == 0),
                            stop=(et == DC - 1),
                        )
                    nc.scalar.activation(
                        out=p_blk[:, mt, :],
                        in_=ps_s,
                        func=mybir.ActivationFunctionType.Exp,
                        scale=1.0 / 32.0,
                    )

                # prefetch + transpose + project the NEXT n-block's q before
                # the long U phase, so the PE never stalls at the boundary
                if ng + 1 < NG:
                    rgb_bf_next = load_rgb_group(ng + 1)[0]
                    qt_cur = emit_qt(rgb_bf_next)

                # U[n, d] + colsum, then normalize and store
                for ntl in range(SUB):
                    n0 = ntl * P
                    ps_u = psup.tile([P, D], F32, tag="psu")
                    ps_cs = psp.tile([P, NBLK], F32, tag="ps")
                    for mc in range(NT):
                        lhs = p_blk[:, mc, n0:n0 + P]
                        nc.tensor.matmul(
                            ps_u[:, 0:NBLK], lhs, freq_bf[:, mc, 0:NBLK],
                            start=(mc == 0), stop=(mc == NT - 1),
                        )
                        nc.tensor.matmul(
                            ps_u[:, NBLK:D], lhs, freq_bf[:, mc, NBLK:D],
                            start=(mc == 0), stop=(mc == NT - 1),
                        )
                        nc.tensor.matmul(
                            ps_cs[:, 0:1], lhs, ones,
                            start=(mc == 0), stop=(mc == NT - 1),
                        )
                    rc = smallp.tile([P, 1], F32, tag="rc")
                    nc.vector.reciprocal(rc, ps_cs[:, 0:1])
                    ot = outp.tile([P, D], F32, tag="ot")
                    # out = (U * (1/colsum)) * 0.5   (fusion weight)
                    nc.vector.tensor_scalar(
                        out=ot, in0=ps_u, scalar1=rc, scalar2=0.5,
                        op0=mybir.AluOpType.mult, op1=mybir.AluOpType.mult,
                    )
                    row0 = ng * NBLK + n0
                    nc.sync.dma_start(out=out[row0:row0 + P, D:2 * D], in_=ot)

    _split_multi_waits(nc)
    return nc


_CACHE: dict = {}


def _get_program() -> bass.Bass:
    if "nc" not in _CACHE:
        _CACHE["nc"] = build_program()
    return _CACHE["nc"]


def _run(in_maps, trace=False, **kw):
    from concourse.bass_utils import run_bass_kernel_spmd

    nc = _get_program()
    return run_bass_kernel_spmd(nc, in_maps, list(range(B)), trace=trace, **kw)


def kernel(rgb, freq, ifreq=None, Wq=None, Wk=None, Wv=None, **_unused):
    rgb = np.asarray(rgb, dtype=np.float32)
    freq = np.asarray(freq, dtype=np.float32)
    Wq = np.ascontiguousarray(np.asarray(Wq, dtype=np.float32))
    Wk = np.ascontiguousarray(np.asarray(Wk, dtype=np.float32))
    in_maps = [
        {
            "rgb": np.ascontiguousarray(rgb[c]),
            "freq": np.ascontiguousarray(freq[c]),
            "Wq": Wq,
            "Wk": Wk,
        }
        for c in range(B)
    ]
    res = _run(in_maps, trace=False)
    return np.stack([res.results[c]["out"] for c in range(B)], axis=0)
